# revision 1
# baseline (speedup 1.0000x reference)
"""nn_GateModLinear on 8 trn2 NeuronCores.

z[b,:] = gW[b,:] * sum_m pW[b,m] * (Ws[m] @ x[b]) + gb[b,:] * (pb @ bs)[b,:]
out = ELU(LayerNorm(z))

Sharding: data-parallel over batch (512 rows/core), Ws replicated.
Per core: fold pW into x per expert (host-precomputed xs[m] = pW[:,m]*x,
0.05% of FLOPs), then accumulate all (m, j) into PSUM on the PE:
  Wx[b,i] = sum_{m,j} xs[m,b,j] * Ws[m,i,j]
bf16 matmuls (rel-err budget 2e-2), fp32 PSUM/LayerNorm.
"""

import numpy as np
import ml_dtypes

B, M, DI, DO = 4096, 8, 2048, 2048
NCORES = 8
BS = B // NCORES  # 512 batch rows per core
LN_EPS = 1e-5
P = 128
JC = DI // P      # 16 contraction chunks of 128
BC = BS // P      # 4 batch chunks of 128
NIH = 2           # output-dim halves
IH = DO // NIH    # 1024
NQ = IH // 512    # 2 psum tiles of 512 per half

BF16 = ml_dtypes.bfloat16

_cache = {}


def _build():
    from contextlib import ExitStack
    import concourse.bacc as bacc
    import concourse.tile as tile
    from concourse import mybir

    f32 = mybir.dt.float32
    bf16 = mybir.dt.bfloat16
    i32 = mybir.dt.int32
    AF = mybir.ActivationFunctionType
    ALU = mybir.AluOpType

    nc = bacc.Bacc("TRN2", target_bir_lowering=False, debug=False, num_devices=1)
    xs_d = nc.dram_tensor("xs", [M, DI, BS], bf16, kind="ExternalInput")
    ws_d = nc.dram_tensor("wsT", [M, DI, DO], bf16, kind="ExternalInput")
    # pbT/bs zero-padded to K=128 on host: K=8 matmuls look idle to the
    # PE activity monitor and drop the clock to 1.2 GHz for ~30 us.
    pb_d = nc.dram_tensor("pbT", [P, BS], f32, kind="ExternalInput")
    bs_d = nc.dram_tensor("bs", [P, DO], f32, kind="ExternalInput")
    gw_d = nc.dram_tensor("gw", [BS, DO], bf16, kind="ExternalInput")
    gb_d = nc.dram_tensor("gb", [BS, DO], bf16, kind="ExternalInput")
    out_d = nc.dram_tensor("out", [BS, DO], bf16, kind="ExternalOutput")

    with ExitStack() as ctx:
        tc = ctx.enter_context(tile.TileContext(nc))
        singles = ctx.enter_context(tc.tile_pool(name="singles", bufs=1))
        ws_pool = ctx.enter_context(tc.tile_pool(name="ws", bufs=8))
        xs_pool = ctx.enter_context(tc.tile_pool(name="xs", bufs=5))
        e_pool = ctx.enter_context(tc.tile_pool(name="elu", bufs=2))
        sm_pool = ctx.enter_context(tc.tile_pool(name="small", bufs=4))
        ps_pool = ctx.enter_context(tc.tile_pool(name="ps", bufs=8, space="PSUM"))

        phases = [(ih, m) for ih in range(NIH) for m in range(M)]

        WCH = 4           # ws jc-chunks per phase
        WJ = JC // WCH    # 4 jc per ws chunk
        XCH = 2           # xs jc-chunks per phase
        XJ = JC // XCH    # 8 jc per xs chunk

        def load(idx):
            ih, m = phases[idx]
            xsrc = xs_d.ap()[m].rearrange("(jc jp) b -> jp jc b", jp=P)
            xts = []
            for h in range(XCH):
                t = xs_pool.tile([P, XJ, BS], bf16, tag="xs",
                                 name=f"xs_{ih}_{m}_{h}")
                nc.gpsimd.dma_start(
                    out=t, in_=xsrc[:, h * XJ:(h + 1) * XJ, :]
                )
                xts.append(t)
            wsrc = ws_d.ap()[m].rearrange("(jc jp) i -> jp jc i", jp=P)
            wst = []
            for h in range(WCH):
                t = ws_pool.tile([P, WJ, IH], bf16, tag="ws",
                                 name=f"ws_{ih}_{m}_{h}")
                nc.sync.dma_start(
                    out=t,
                    in_=wsrc[:, h * WJ:(h + 1) * WJ,
                             ih * IH:(ih + 1) * IH],
                )
                wst.append(t)
            return xts, wst

        # ---- PE warm-up: full-K dummy matmuls with no DMA deps keep the
        # PE array-utilization monitor busy so the clock ramps to 2.4 GHz
        # before the real stream starts ----
        wl = singles.tile([P, P], bf16)
        nc.vector.memset(wl, 1.0)
        wr = singles.tile([P, 512], bf16)
        nc.vector.memset(wr, 0.5)
        wp = ps_pool.tile([P, 512], f32, tag="acc", name="warm")
        for _ in range(12):
            nc.tensor.matmul(wp, wl, wr, start=True, stop=True)

        # ---- bias operands early on the sync queue (tiny, ahead of ws) ----
        pbT = singles.tile([P, BS], f32)
        nc.sync.dma_start(out=pbT, in_=pb_d.ap())
        bs_sb = singles.tile([P, DO], f32)
        nc.sync.dma_start(out=bs_sb, in_=bs_d.ap())

        # prefetch phase 0 (xs on gpsimd queue, ws on sync queue)
        pending = load(0)

        gb = singles.tile([P, BC, DO], bf16)
        nc.scalar.dma_start(
            out=gb, in_=gb_d.ap().rearrange("(bc p) i -> p bc i", p=P)
        )
        z = singles.tile([P, BC, DO], f32)

        # ---- bias: z = pb @ bs (drained via ACT copy — no gb dependency,
        # so psum slots recycle fast); gb multiply applied below once loaded
        for bc in range(BC):
            for q in range(DO // 512):
                bps = ps_pool.tile([P, 512], f32, tag="acc", name=f"bps_{bc}_{q}")
                nc.tensor.matmul(
                    bps,
                    pbT[:, bc * P:(bc + 1) * P],
                    bs_sb[:, q * 512:(q + 1) * 512],
                    start=True, stop=True,
                )
                if q % 2 == 0:
                    nc.scalar.copy(z[:, bc, q * 512:(q + 1) * 512], bps)
                else:
                    nc.vector.tensor_copy(z[:, bc, q * 512:(q + 1) * 512], bps)
        for bc in range(BC):
            nc.vector.tensor_mul(z[:, bc, :], z[:, bc, :], gb[:, bc, :])

        gw = singles.tile([P, BC, DO], bf16)
        nc.scalar.dma_start(
            out=gw, in_=gw_d.ap().rearrange("(bc p) i -> p bc i", p=P)
        )

        out_ap = out_d.ap().rearrange("(bc p) i -> p bc i", p=P)

        def drain(ih, acc, bc):
            for q in range(NQ):
                i0 = ih * IH + q * 512
                nc.vector.tensor_mul(acc[bc][q], acc[bc][q],
                                     gw[:, bc, i0:i0 + 512])
                nc.vector.tensor_add(z[:, bc, i0:i0 + 512],
                                     z[:, bc, i0:i0 + 512], acc[bc][q])

        def epilogue(bc):
            # LayerNorm + ELU + store for batch chunk bc
            row = z[:, bc, :]
            stats = sm_pool.tile([P, DO // 512, 6], f32, tag="stats",
                                 name=f"stats_{bc}")
            for s in range(DO // 512):
                nc.vector.bn_stats(out=stats[:, s, :],
                                   in_=row[:, s * 512:(s + 1) * 512])
            mv = sm_pool.tile([P, 2], f32, tag="mv", name=f"mv_{bc}")
            nc.vector.bn_aggr(out=mv, in_=stats)
            # rstd = 1/sqrt(var+eps) on DVE (bitcast seed + 2 Newton steps)
            # so the ACT engine's function table stays pinned to Exp.
            rstd = sm_pool.tile([P, 1], f32, tag="rstd", name=f"rstd_{bc}")
            ve = sm_pool.tile([P, 1], f32, tag="ve", name=f"ve_{bc}")
            nc.vector.tensor_scalar_add(ve, mv[:, 1:2], LN_EPS)  # v + eps
            vh = sm_pool.tile([P, 1], f32, tag="vh", name=f"vh_{bc}")
            nc.vector.tensor_scalar_mul(vh, ve, 0.5)
            # seed bits: 0x5f3759df - (i >> 1)  ==  ((i>>1) ^ ~0) + 0x5f3759e0
            nc.vector.tensor_scalar(
                rstd.bitcast(i32), ve.bitcast(i32), 1, -1,
                op0=ALU.logical_shift_right, op1=ALU.bitwise_xor)
            nc.vector.tensor_scalar_add(rstd.bitcast(i32), rstd.bitcast(i32),
                                        0x5f3759e0)
            for _ in range(1):  # y *= 1.5 - vh*y*y
                t1 = sm_pool.tile([P, 1], f32, tag="t1", name=f"t1_{bc}")
                nc.vector.tensor_mul(t1, rstd, rstd)
                nc.vector.tensor_mul(t1, t1, vh)
                nc.vector.tensor_scalar(t1, t1, -1.0, -1.5,
                                        op0=ALU.mult, op1=ALU.subtract)
                nc.vector.tensor_mul(rstd, rstd, t1)
            nmu = sm_pool.tile([P, 1], f32, tag="nmu", name=f"nmu_{bc}")
            nc.vector.tensor_scalar_mul(nmu, mv[:, 0:1], -1.0)
            nmr = sm_pool.tile([P, 1], f32, tag="nmr", name=f"nmr_{bc}")
            nc.vector.tensor_mul(nmr, nmu, rstd)
            # per output half: et = exp(y) on ACT || y on DVE, then fuse+store
            for h in range(2):
                hs = slice(h * (DO // 2), (h + 1) * (DO // 2))
                rh = row[:, hs]
                et = e_pool.tile([P, DO // 2], bf16, tag="et", name=f"et_{bc}_{h}")
                nc.scalar.activation(out=et, in_=rh, func=AF.Exp,
                                     bias=nmr, scale=rstd)
                yt = e_pool.tile([P, DO // 2], bf16, tag="yt", name=f"yt_{bc}_{h}")
                nc.vector.tensor_scalar(yt, rh, nmu, rstd,
                                        op0=ALU.add, op1=ALU.mult)
                nc.vector.tensor_scalar(et, et, -1.0, 0.0,
                                        op0=ALU.add, op1=ALU.min)
                ot = e_pool.tile([P, DO // 2], bf16, tag="ot", name=f"ot_{bc}_{h}")
                nc.vector.scalar_tensor_tensor(ot, yt, 0.0, et,
                                               op0=ALU.max, op1=ALU.add)
                eng = nc.sync if bc == BC - 1 else nc.gpsimd
                eng.dma_start(out=out_ap[:, bc, hs], in_=ot)

        # ---- main accumulation ----
        for idx, (ih, m) in enumerate(phases):
            xts, wst = pending
            if idx + 1 < len(phases):
                pending = load(idx + 1)
            if m == 0:
                acc = [[ps_pool.tile([P, 512], f32, tag="acc",
                                     name=f"acc_{ih}_{bc}_{q}")
                        for q in range(NQ)] for bc in range(BC)]
            last = (m == M - 1)
            if last:
                # bc-major so each chunk finishes early and its epilogue
                # overlaps the remaining chunks' matmuls
                for bc in range(BC):
                    for jc in range(JC):
                        xt = xts[jc // XJ]
                        w = wst[jc // WJ]
                        for q in range(NQ):
                            nc.tensor.matmul(
                                acc[bc][q],
                                xt[:, jc % XJ, bc * P:(bc + 1) * P],
                                w[:, jc % WJ, q * 512:(q + 1) * 512],
                                start=False,
                                stop=(jc == JC - 1),
                            )
                    drain(ih, acc, bc)
                    if ih == NIH - 1:
                        epilogue(bc)
            else:
                for jc in range(JC):
                    xt = xts[jc // XJ]
                    w = wst[jc // WJ]
                    for bc in range(BC):
                        for q in range(NQ):
                            nc.tensor.matmul(
                                acc[bc][q],
                                xt[:, jc % XJ, bc * P:(bc + 1) * P],
                                w[:, jc % WJ, q * 512:(q + 1) * 512],
                                start=(m == 0 and jc == 0),
                                stop=False,
                            )

    nc.compile()
    return nc


def _prep_inputs(x, Ws, bs, pW, pb, gW, gb):
    x = np.asarray(x, np.float32)
    pW = np.asarray(pW, np.float32)
    # xs[m, j, b] = pW[b, m] * x[b, j], bf16, per-core column slices
    xT = np.ascontiguousarray(x.T)                        # [DI, B]
    wsT = np.ascontiguousarray(
        np.asarray(Ws, np.float32).transpose(0, 2, 1)
    ).astype(BF16)                                        # [M, DI, DO]
    pbT = np.zeros((128, B), np.float32)                  # K=128 zero-pad
    pbT[:M] = np.asarray(pb, np.float32).T
    bs_pad = np.zeros((128, DO), np.float32)
    bs_pad[:M] = np.asarray(bs, np.float32)
    gW16 = np.asarray(gW, np.float32).astype(BF16)
    gb16 = np.asarray(gb, np.float32).astype(BF16)
    in_maps = []
    for c in range(NCORES):
        sl = slice(c * BS, (c + 1) * BS)
        xs = (pW[sl].T[:, None, :] * xT[None, :, sl]).astype(BF16)
        in_maps.append({
            "xs": np.ascontiguousarray(xs),               # [M, DI, BS]
            "wsT": wsT,
            "pbT": np.ascontiguousarray(pbT[:, sl]),
            "bs": bs_pad,
            "gw": np.ascontiguousarray(gW16[sl]),
            "gb": np.ascontiguousarray(gb16[sl]),
        })
    return in_maps


def kernel(x, Ws, bs, pW, pb, gW, gb, _trace=False, _tmpdir=None):
    from concourse import bass_utils

    if "nc" not in _cache:
        _cache["nc"] = _build()
    nc = _cache["nc"]
    in_maps = _prep_inputs(x, Ws, bs, pW, pb, gW, gb)
    res = bass_utils.run_bass_kernel_spmd(
        nc, in_maps, core_ids=list(range(NCORES)),
        trace=_trace, tmpdir=_tmpdir,
    )
    _cache["last_result"] = res
    out = np.concatenate([res.results[c]["out"] for c in range(NCORES)], axis=0)
    return np.asarray(out, dtype=np.float32)



# revision 4
# speedup vs baseline: 1.0481x; 1.0481x over previous
"""nn_GateModLinear on 8 trn2 NeuronCores.

z[b,:] = gW[b,:] * sum_m pW[b,m] * (Ws[m] @ x[b]) + gb[b,:] * (pb @ bs)[b,:]
out = ELU(LayerNorm(z))

Sharding: data-parallel over batch (512 rows/core), Ws replicated.
Per core: fold pW into x per expert (host-precomputed xs[m] = pW[:,m]*x,
0.05% of FLOPs), then accumulate all (m, j) into PSUM on the PE:
  Wx[b,i] = sum_{m,j} xs[m,b,j] * Ws[m,i,j]
bf16 matmuls (rel-err budget 2e-2), fp32 PSUM accumulate.

The bias path zb = gb*(pb@bs) is fully precomputed on host (it is 0.003%
of the FLOPs) and streamed as bf16 — no fp32 bias matmuls on the PE.
Epilogue: drain = acc*gw on DVE (PSUM->SBUF bf16), += zb on DVE 2x mode,
LN stats per 512-chunk, ELU as ACT Relu(r*z+b) + ACT Exp(r*z+b) with
cheap DVE fixups (tensor_scalar 4x / tensor_tensor 2x only).
"""

import numpy as np
import ml_dtypes

B, M, DI, DO = 4096, 8, 2048, 2048
NCORES = 8
BS = B // NCORES  # 512 batch rows per core
LN_EPS = 1e-5
P = 128
JC = DI // P      # 16 contraction chunks of 128
BC = BS // P      # 4 batch chunks of 128
NIH = 2           # output-dim halves
IH = DO // NIH    # 1024
NQ = IH // 512    # 2 psum tiles of 512 per half

BF16 = ml_dtypes.bfloat16

_cache = {}


def _build():
    from contextlib import ExitStack
    import concourse.bacc as bacc
    import concourse.tile as tile
    from concourse import mybir

    f32 = mybir.dt.float32
    bf16 = mybir.dt.bfloat16
    i32 = mybir.dt.int32
    AF = mybir.ActivationFunctionType
    ALU = mybir.AluOpType

    nc = bacc.Bacc("TRN2", target_bir_lowering=False, debug=False, num_devices=1)
    xs_d = nc.dram_tensor("xs", [M, DI, BS], bf16, kind="ExternalInput")
    ws_d = nc.dram_tensor("wsT", [M, DI, DO], bf16, kind="ExternalInput")
    zb_d = nc.dram_tensor("zb", [BS, DO], bf16, kind="ExternalInput")
    gw_d = nc.dram_tensor("gw", [BS, DO], bf16, kind="ExternalInput")
    out_d = nc.dram_tensor("out", [BS, DO], bf16, kind="ExternalOutput")

    with ExitStack() as ctx:
        tc = ctx.enter_context(tile.TileContext(nc))
        singles = ctx.enter_context(tc.tile_pool(name="singles", bufs=1))
        ws_pool = ctx.enter_context(tc.tile_pool(name="ws", bufs=18))
        xs_pool = ctx.enter_context(tc.tile_pool(name="xs", bufs=10))
        e_pool = ctx.enter_context(tc.tile_pool(name="elu", bufs=3))
        sm_pool = ctx.enter_context(tc.tile_pool(name="small", bufs=4))
        ps_pool = ctx.enter_context(tc.tile_pool(name="ps", bufs=8, space="PSUM"))

        phases = [(ih, m) for ih in range(NIH) for m in range(M)]

        WCH = 8           # ws jc-chunks per phase (0.5 MB each)
        WJ = JC // WCH    # 2 jc per ws chunk
        XCH = 4           # xs jc-chunks per phase (0.5 MB each)
        XJ = JC // XCH    # 4 jc per xs chunk

        def load(idx):
            ih, m = phases[idx]
            xsrc = xs_d.ap()[m].rearrange("(jc jp) b -> jp jc b", jp=P)
            xts = []
            for h in range(XCH):
                t = xs_pool.tile([P, XJ, BS], bf16, tag="xs",
                                 name=f"xs_{ih}_{m}_{h}")
                # first xs chunk of the whole program goes on the scalar
                # HWDGE queue so it lands in parallel with ws chunk 0 on
                # sync — the stream's first matmul needs both.
                eng = nc.scalar if (idx == 0 and h == 0) else nc.gpsimd
                eng.dma_start(out=t, in_=xsrc[:, h * XJ:(h + 1) * XJ, :])
                xts.append(t)
            wsrc = ws_d.ap()[m].rearrange("(jc jp) i -> jp jc i", jp=P)
            wst = []
            for h in range(WCH):
                t = ws_pool.tile([P, WJ, IH], bf16, tag="ws",
                                 name=f"ws_{ih}_{m}_{h}")
                nc.sync.dma_start(
                    out=t,
                    in_=wsrc[:, h * WJ:(h + 1) * WJ,
                             ih * IH:(ih + 1) * IH],
                )
                wst.append(t)
            return xts, wst

        # ---- PE warm-up: short N=128 matmuls with no DMA deps keep the
        # PE activity monitor busy (so the clock is at 2.4 GHz when the
        # real stream starts) and bridge the first-chunk DMA latency ----
        wl = singles.tile([P, P], bf16)
        nc.vector.memset(wl, 1.0)
        wr = singles.tile([P, P], bf16)
        nc.vector.memset(wr, 0.5)
        wp = ps_pool.tile([P, 512], f32, tag="acc", name="warm")
        for _ in range(30):
            nc.tensor.matmul(wp[:, 0:P], wl, wr, start=True, stop=True)

        # prefetch phases 0 and 1 up front (ws on sync, xs on gpsimd)
        pending = [load(0), load(1)]

        # zb/gw ride the sync HWDGE ring behind later phases' ws chunks:
        # ring FIFO order paces their transfers after the phase-3..6 ws
        # data, so they never compete with the phase-0/1 operand stream.
        zb = singles.tile([P, BC, DO], bf16)
        gw = singles.tile([P, BC, DO], bf16)
        zb_src = zb_d.ap().rearrange("(bc p) i -> p bc i", p=P)
        gw_src = gw_d.ap().rearrange("(bc p) i -> p bc i", p=P)

        z = singles.tile([P, BC, DO], bf16)
        stats = singles.tile([P, BC, DO // 512, 6], f32)

        out_ap = out_d.ap().rearrange("(bc p) i -> p bc i", p=P)

        def drain_chunk(ih, acc, bc, q):
            i0 = ih * IH + q * 512
            zs = z[:, bc, i0:i0 + 512]
            # z = acc * gw  (DVE 1x, PSUM source)
            nc.vector.tensor_mul(zs, acc[bc][q], gw[:, bc, i0:i0 + 512])
            # z += zb       (DVE 2x, all bf16 SBUF)
            nc.vector.tensor_add(zs, zs, zb[:, bc, i0:i0 + 512])
            # LN partial stats for this 512-chunk
            nc.vector.bn_stats(out=stats[:, bc, (i0 // 512), :], in_=zs)

        def epilogue(bc):
            # LayerNorm + ELU + store for batch chunk bc
            mv = sm_pool.tile([P, 2], f32, tag="mv", name=f"mv_{bc}")
            nc.vector.bn_aggr(out=mv, in_=stats[:, bc])
            # rstd = 1/sqrt(var+eps) on DVE (bitcast seed + 1 Newton step)
            # so the ACT engine's function table never switches sets.
            rstd = sm_pool.tile([P, 1], f32, tag="rstd", name=f"rstd_{bc}")
            ve = sm_pool.tile([P, 1], f32, tag="ve", name=f"ve_{bc}")
            nc.vector.tensor_scalar_add(ve, mv[:, 1:2], LN_EPS)  # v + eps
            vh = sm_pool.tile([P, 1], f32, tag="vh", name=f"vh_{bc}")
            nc.vector.tensor_scalar_mul(vh, ve, 0.5)
            # seed bits: 0x5f3759df - (i >> 1)  ==  ((i>>1) ^ ~0) + 0x5f3759e0
            nc.vector.tensor_scalar(
                rstd.bitcast(i32), ve.bitcast(i32), 1, -1,
                op0=ALU.logical_shift_right, op1=ALU.bitwise_xor)
            nc.vector.tensor_scalar_add(rstd.bitcast(i32), rstd.bitcast(i32),
                                        0x5f3759e0)
            for _ in range(2):  # y *= 1.5 - vh*y*y
                t1 = sm_pool.tile([P, 1], f32, tag="t1", name=f"t1_{bc}")
                nc.vector.tensor_mul(t1, rstd, rstd)
                nc.vector.tensor_mul(t1, t1, vh)
                nc.vector.tensor_scalar(t1, t1, -1.0, -1.5,
                                        op0=ALU.mult, op1=ALU.subtract)
                nc.vector.tensor_mul(rstd, rstd, t1)
            nmr = sm_pool.tile([P, 1], f32, tag="nmr", name=f"nmr_{bc}")
            nc.vector.tensor_mul(nmr, mv[:, 0:1], rstd)
            nc.vector.tensor_scalar_mul(nmr, nmr, -1.0)
            # per output half: y = rstd*z + nmr;
            # ELU(y) = max(y,0) + min(exp(y)-1, 0)
            for h in range(2):
                hs = slice(h * (DO // 2), (h + 1) * (DO // 2))
                rh = z[:, bc, hs]
                rel = e_pool.tile([P, DO // 2], bf16, tag="rel",
                                  name=f"rel_{bc}_{h}")
                nc.scalar.activation(out=rel, in_=rh, func=AF.Relu,
                                     bias=nmr, scale=rstd)
                et = e_pool.tile([P, DO // 2], bf16, tag="et",
                                 name=f"et_{bc}_{h}")
                nc.scalar.activation(out=et, in_=rh, func=AF.Exp,
                                     bias=nmr, scale=rstd)
                nc.vector.tensor_scalar(et, et, -1.0, 0.0,
                                        op0=ALU.add, op1=ALU.min)
                ot = e_pool.tile([P, DO // 2], bf16, tag="ot",
                                 name=f"ot_{bc}_{h}")
                nc.vector.tensor_add(ot, rel, et)
                nc.sync.dma_start(out=out_ap[:, bc, hs], in_=ot)

        # ---- main accumulation ----
        for idx, (ih, m) in enumerate(phases):
            xts, wst = pending.pop(0)
            if idx + 2 < len(phases):
                pending.append(load(idx + 2))
            if 1 <= idx <= BC:
                bc = idx - 1
                nc.sync.dma_start(out=zb[:, bc, :], in_=zb_src[:, bc, :])
                nc.sync.dma_start(out=gw[:, bc, :], in_=gw_src[:, bc, :])
            if m == 0:
                acc = [[ps_pool.tile([P, 512], f32, tag="acc",
                                     name=f"acc_{ih}_{bc}_{q}")
                        for q in range(NQ)] for bc in range(BC)]
            last = (m == M - 1)
            if last:
                # bc-major so each chunk finishes early and its drain +
                # epilogue overlap the remaining chunks' matmuls
                for bc in range(BC):
                    for jc in range(JC):
                        xt = xts[jc // XJ]
                        w = wst[jc // WJ]
                        for q in range(NQ):
                            nc.tensor.matmul(
                                acc[bc][q],
                                xt[:, jc % XJ, bc * P:(bc + 1) * P],
                                w[:, jc % WJ, q * 512:(q + 1) * 512],
                                start=False,
                                stop=(jc == JC - 1),
                            )
                    for q in range(NQ):
                        drain_chunk(ih, acc, bc, q)
                    if ih == NIH - 1:
                        epilogue(bc)
            else:
                for jc in range(JC):
                    xt = xts[jc // XJ]
                    w = wst[jc // WJ]
                    for bc in range(BC):
                        for q in range(NQ):
                            nc.tensor.matmul(
                                acc[bc][q],
                                xt[:, jc % XJ, bc * P:(bc + 1) * P],
                                w[:, jc % WJ, q * 512:(q + 1) * 512],
                                start=(m == 0 and jc == 0),
                                stop=False,
                            )

    nc.compile()
    return nc


def _prep_inputs(x, Ws, bs, pW, pb, gW, gb):
    x = np.asarray(x, np.float32)
    pW = np.asarray(pW, np.float32)
    # xs[m, j, b] = pW[b, m] * x[b, j], bf16, per-core column slices
    xT = np.ascontiguousarray(x.T)                        # [DI, B]
    wsT = np.ascontiguousarray(
        np.asarray(Ws, np.float32).transpose(0, 2, 1)
    ).astype(BF16)                                        # [M, DI, DO]
    # bias path entirely on host: zb = gb * (pb @ bs)    [B, DO]
    zb = (np.asarray(gb, np.float32)
          * (np.asarray(pb, np.float32) @ np.asarray(bs, np.float32)))
    zb16 = zb.astype(BF16)
    gW16 = np.asarray(gW, np.float32).astype(BF16)
    in_maps = []
    for c in range(NCORES):
        sl = slice(c * BS, (c + 1) * BS)
        xs = (pW[sl].T[:, None, :] * xT[None, :, sl]).astype(BF16)
        in_maps.append({
            "xs": np.ascontiguousarray(xs),               # [M, DI, BS]
            "wsT": wsT,
            "zb": np.ascontiguousarray(zb16[sl]),
            "gw": np.ascontiguousarray(gW16[sl]),
        })
    return in_maps


def kernel(x, Ws, bs, pW, pb, gW, gb, _trace=False, _tmpdir=None):
    from concourse import bass_utils

    if "nc" not in _cache:
        _cache["nc"] = _build()
    nc = _cache["nc"]
    in_maps = _prep_inputs(x, Ws, bs, pW, pb, gW, gb)
    res = bass_utils.run_bass_kernel_spmd(
        nc, in_maps, core_ids=list(range(NCORES)),
        trace=_trace, tmpdir=_tmpdir,
    )
    _cache["last_result"] = res
    out = np.concatenate([res.results[c]["out"] for c in range(NCORES)], axis=0)
    return np.asarray(out, dtype=np.float32)


# revision 7
# speedup vs baseline: 1.0481x; 1.0000x over previous
"""nn_GateModLinear on 8 trn2 NeuronCores.

z[b,:] = gW[b,:] * sum_m pW[b,m] * (Ws[m] @ x[b]) + gb[b,:] * (pb @ bs)[b,:]
out = ELU(LayerNorm(z))

Sharding: data-parallel over batch (512 rows/core), Ws replicated.
Per core: fold pW into x per expert (host-precomputed xs[m] = pW[:,m]*x,
0.05% of FLOPs), then accumulate all (m, j) into PSUM on the PE:
  Wx[b,i] = sum_{m,j} xs[m,b,j] * Ws[m,i,j]
bf16 matmuls (rel-err budget 2e-2), fp32 PSUM accumulate.

The bias path zb = gb*(pb@bs) is fully precomputed on host (0.003% of
the FLOPs) and streamed as bf16 — no fp32 bias matmuls on the PE.
Epilogue: drain = acc*gw on DVE (PSUM->SBUF bf16), += zb on DVE 2x mode,
LN stats per 512-chunk, fused 8-op Newton rsqrt (magic seed adjusted to
read v/2 directly), ELU as ACT Relu(r*z+b) + ACT Exp(r*z+b) with cheap
DVE/GpSimd fixups. Last phases drain q-major so each PSUM bank's chain
starts as early as possible.
"""

import numpy as np
import ml_dtypes

B, M, DI, DO = 4096, 8, 2048, 2048
NCORES = 8
BS = B // NCORES  # 512 batch rows per core
LN_EPS = 1e-5
P = 128
JC = DI // P      # 16 contraction chunks of 128
BC = BS // P      # 4 batch chunks of 128
NIH = 2           # output-dim halves
IH = DO // NIH    # 1024
NQ = IH // 512    # 2 psum tiles of 512 per half

BF16 = ml_dtypes.bfloat16

_cache = {}


def _build():
    from contextlib import ExitStack
    import concourse.bacc as bacc
    import concourse.tile as tile
    from concourse import mybir

    f32 = mybir.dt.float32
    bf16 = mybir.dt.bfloat16
    i32 = mybir.dt.int32
    AF = mybir.ActivationFunctionType
    ALU = mybir.AluOpType

    nc = bacc.Bacc("TRN2", target_bir_lowering=False, debug=False, num_devices=1)
    xs_d = nc.dram_tensor("xs", [M, DI, BS], bf16, kind="ExternalInput")
    ws_d = nc.dram_tensor("wsT", [M, DI, DO], bf16, kind="ExternalInput")
    zb_d = nc.dram_tensor("zb", [BS, DO], bf16, kind="ExternalInput")
    gw_d = nc.dram_tensor("gw", [BS, DO], bf16, kind="ExternalInput")
    out_d = nc.dram_tensor("out", [BS, DO], bf16, kind="ExternalOutput")

    with ExitStack() as ctx:
        tc = ctx.enter_context(tile.TileContext(nc))
        singles = ctx.enter_context(tc.tile_pool(name="singles", bufs=1))
        ws_pool = ctx.enter_context(tc.tile_pool(name="ws", bufs=18))
        xs_pool = ctx.enter_context(tc.tile_pool(name="xs", bufs=10))
        e_pool = ctx.enter_context(tc.tile_pool(name="elu", bufs=3))
        sm_pool = ctx.enter_context(tc.tile_pool(name="small", bufs=4))
        ps_pool = ctx.enter_context(tc.tile_pool(name="ps", bufs=8, space="PSUM"))

        phases = [(ih, m) for ih in range(NIH) for m in range(M)]

        def load(idx):
            ih, m = phases[idx]
            # jc-chunk splits; phase 0 leads with small chunks so the
            # first matmul's operands land as early as possible.
            if idx == 0:
                xsplit = [("x0", 2), ("x0", 2), ("xs", 4), ("xs", 4), ("xs", 4)]
                wsplit = [("w0", 1), ("w0", 1)] + [("ws", 2)] * 7
            else:
                xsplit = [("xs", 4)] * 4
                wsplit = [("ws", 2)] * 8
            xsrc = xs_d.ap()[m].rearrange("(jc jp) b -> jp jc b", jp=P)
            xmap = []
            off = 0
            for h, (tg, nj) in enumerate(xsplit):
                t = xs_pool.tile([P, nj, BS], bf16, tag=tg,
                                 bufs=2 if tg == "x0" else None,
                                 name=f"xs_{ih}_{m}_{h}")
                # the very first xs chunk rides the scalar HWDGE queue so
                # it lands in parallel with ws chunk 0 on the sync queue.
                eng = nc.scalar if (idx == 0 and h == 0) else nc.gpsimd
                eng.dma_start(out=t, in_=xsrc[:, off:off + nj, :])
                for j in range(nj):
                    xmap.append((t, j))
                off += nj
            wsrc = ws_d.ap()[m].rearrange("(jc jp) i -> jp jc i", jp=P)
            wmap = []
            off = 0
            for h, (tg, nj) in enumerate(wsplit):
                t = ws_pool.tile([P, nj, IH], bf16, tag=tg,
                                 bufs=2 if tg == "w0" else None,
                                 name=f"ws_{ih}_{m}_{h}")
                nc.sync.dma_start(
                    out=t,
                    in_=wsrc[:, off:off + nj, ih * IH:(ih + 1) * IH],
                )
                for j in range(nj):
                    wmap.append((t, j))
                off += nj
            return xmap, wmap

        # ---- PE warm-up: short N=128 matmuls with no DMA deps keep the
        # PE activity monitor busy (clock at 2.4 GHz when the stream
        # starts) and bridge the first-chunk DMA latency ----
        wl = singles.tile([P, P], bf16)
        nc.vector.memset(wl, 1.0)
        wr = singles.tile([P, P], bf16)
        nc.vector.memset(wr, 0.5)
        wp = ps_pool.tile([P, 512], f32, tag="acc", name="warm")
        for _ in range(30):
            nc.tensor.matmul(wp[:, 0:P], wl, wr, start=True, stop=True)

        # prefetch phases 0 and 1 up front (ws on sync, xs on gpsimd)
        pending = [load(0), load(1)]

        # zb/gw ride the sync HWDGE ring behind later phases' ws chunks:
        # ring FIFO order paces their transfers after the phase-3..6 ws
        # data, so they never compete with the phase-0/1 operand stream.
        zb = singles.tile([P, BC, DO], bf16)
        gw = singles.tile([P, BC, DO], bf16)
        zb_src = zb_d.ap().rearrange("(bc p) i -> p bc i", p=P)
        gw_src = gw_d.ap().rearrange("(bc p) i -> p bc i", p=P)

        z = singles.tile([P, BC, DO], bf16)
        stats = singles.tile([P, BC, DO // 512, 6], f32)

        out_ap = out_d.ap().rearrange("(bc p) i -> p bc i", p=P)

        def drain_chunk(ih, acc, bc, q):
            i0 = ih * IH + q * 512
            zs = z[:, bc, i0:i0 + 512]
            # z = acc * gw  (DVE 1x, PSUM source)
            nc.vector.tensor_mul(zs, acc[bc][q], gw[:, bc, i0:i0 + 512])
            # z += zb       (DVE 2x, all bf16 SBUF)
            nc.vector.tensor_add(zs, zs, zb[:, bc, i0:i0 + 512])
            # LN partial stats for this 512-chunk
            nc.vector.bn_stats(out=stats[:, bc, (i0 // 512), :], in_=zs)

        def epilogue(bc):
            # LayerNorm + ELU + store for batch chunk bc
            mv = sm_pool.tile([P, 2], f32, tag="mv", name=f"mv_{bc}")
            nc.vector.bn_aggr(out=mv, in_=stats[:, bc])
            # rstd = 1/sqrt(var+eps) via bitcast seed + 1 Newton step on
            # DVE (the ACT table never switches sets). The seed constant
            # is adjusted to read vh = (var+eps)/2 directly:
            #   bits(1/sqrt(2*vh)) ~ 0x5ef759df - (bits(vh) >> 1)
            vh = sm_pool.tile([P, 1], f32, tag="vh", name=f"vh_{bc}")
            nc.vector.tensor_scalar(vh, mv[:, 1:2], 0.5, 0.5 * LN_EPS,
                                    op0=ALU.mult, op1=ALU.add)
            rstd = sm_pool.tile([P, 1], f32, tag="rstd", name=f"rstd_{bc}")
            nc.vector.tensor_scalar(
                rstd.bitcast(i32), vh.bitcast(i32), 1, -1,
                op0=ALU.logical_shift_right, op1=ALU.bitwise_xor)
            nc.vector.tensor_scalar_add(rstd.bitcast(i32), rstd.bitcast(i32),
                                        0x5ef759e0)
            # y *= 1.5 - vh*y*y
            t1 = sm_pool.tile([P, 1], f32, tag="t1", name=f"t1_{bc}")
            nc.vector.tensor_mul(t1, rstd, rstd)
            nc.vector.tensor_mul(t1, t1, vh)
            nc.vector.tensor_scalar(t1, t1, -1.0, -1.5,
                                    op0=ALU.mult, op1=ALU.subtract)
            nc.vector.tensor_mul(rstd, rstd, t1)
            nmr = sm_pool.tile([P, 1], f32, tag="nmr", name=f"nmr_{bc}")
            nc.vector.scalar_tensor_tensor(nmr, mv[:, 0:1], -1.0, rstd,
                                           op0=ALU.mult, op1=ALU.mult)
            # per output half: y = rstd*z + nmr;
            # ELU(y) = max(y,0) + min(exp(y)-1, 0)
            for h in range(2):
                hs = slice(h * (DO // 2), (h + 1) * (DO // 2))
                rh = z[:, bc, hs]
                rel = e_pool.tile([P, DO // 2], bf16, tag="rel",
                                  name=f"rel_{bc}_{h}")
                nc.scalar.activation(out=rel, in_=rh, func=AF.Relu,
                                     bias=nmr, scale=rstd)
                et = e_pool.tile([P, DO // 2], bf16, tag="et",
                                 name=f"et_{bc}_{h}")
                nc.scalar.activation(out=et, in_=rh, func=AF.Exp,
                                     bias=nmr, scale=rstd)
                nc.vector.tensor_scalar(et, et, -1.0, 0.0,
                                        op0=ALU.add, op1=ALU.min)
                ot = e_pool.tile([P, DO // 2], bf16, tag="ot",
                                 name=f"ot_{bc}_{h}")
                # keep the tail chunk's critical path on the fast DVE;
                # earlier chunks offload the add to the idle GpSimd
                eng = nc.vector if bc == BC - 1 else nc.gpsimd
                eng.tensor_add(ot, rel, et)
                nc.sync.dma_start(out=out_ap[:, bc, hs], in_=ot)

        # ---- main accumulation ----
        for idx, (ih, m) in enumerate(phases):
            xmap, wmap = pending.pop(0)
            if idx + 2 < len(phases):
                pending.append(load(idx + 2))
            if 1 <= idx <= BC:
                bc = idx - 1
                nc.sync.dma_start(out=zb[:, bc, :], in_=zb_src[:, bc, :])
                nc.sync.dma_start(out=gw[:, bc, :], in_=gw_src[:, bc, :])
            if m == 0:
                acc = [[ps_pool.tile([P, 512], f32, tag="acc",
                                     name=f"acc_{ih}_{bc}_{q}")
                        for q in range(NQ)] for bc in range(BC)]
            last = (m == M - 1)
            if last:
                # bc-major, q-major: each PSUM bank finishes its 16-MM
                # run early so drains/stats/epilogue overlap the
                # remaining matmuls
                for bc in range(BC):
                    for q in range(NQ):
                        for jc in range(JC):
                            xt, xj = xmap[jc]
                            w, wj = wmap[jc]
                            nc.tensor.matmul(
                                acc[bc][q],
                                xt[:, xj, bc * P:(bc + 1) * P],
                                w[:, wj, q * 512:(q + 1) * 512],
                                start=False,
                                stop=(jc == JC - 1),
                            )
                        drain_chunk(ih, acc, bc, q)
                    if ih == NIH - 1:
                        epilogue(bc)
            else:
                for jc in range(JC):
                    xt, xj = xmap[jc]
                    w, wj = wmap[jc]
                    for bc in range(BC):
                        for q in range(NQ):
                            nc.tensor.matmul(
                                acc[bc][q],
                                xt[:, xj, bc * P:(bc + 1) * P],
                                w[:, wj, q * 512:(q + 1) * 512],
                                start=(m == 0 and jc == 0),
                                stop=False,
                            )

    nc.compile()
    return nc


def _prep_inputs(x, Ws, bs, pW, pb, gW, gb):
    x = np.asarray(x, np.float32)
    pW = np.asarray(pW, np.float32)
    # xs[m, j, b] = pW[b, m] * x[b, j], bf16, per-core column slices
    xT = np.ascontiguousarray(x.T)                        # [DI, B]
    wsT = np.ascontiguousarray(
        np.asarray(Ws, np.float32).transpose(0, 2, 1)
    ).astype(BF16)                                        # [M, DI, DO]
    # bias path entirely on host: zb = gb * (pb @ bs)    [B, DO]
    zb = (np.asarray(gb, np.float32)
          * (np.asarray(pb, np.float32) @ np.asarray(bs, np.float32)))
    zb16 = zb.astype(BF16)
    gW16 = np.asarray(gW, np.float32).astype(BF16)
    in_maps = []
    for c in range(NCORES):
        sl = slice(c * BS, (c + 1) * BS)
        xs = (pW[sl].T[:, None, :] * xT[None, :, sl]).astype(BF16)
        in_maps.append({
            "xs": np.ascontiguousarray(xs),               # [M, DI, BS]
            "wsT": wsT,
            "zb": np.ascontiguousarray(zb16[sl]),
            "gw": np.ascontiguousarray(gW16[sl]),
        })
    return in_maps


def kernel(x, Ws, bs, pW, pb, gW, gb, _trace=False, _tmpdir=None):
    from concourse import bass_utils

    if "nc" not in _cache:
        _cache["nc"] = _build()
    nc = _cache["nc"]
    in_maps = _prep_inputs(x, Ws, bs, pW, pb, gW, gb)
    res = bass_utils.run_bass_kernel_spmd(
        nc, in_maps, core_ids=list(range(NCORES)),
        trace=_trace, tmpdir=_tmpdir,
    )
    _cache["last_result"] = res
    out = np.concatenate([res.results[c]["out"] for c in range(NCORES)], axis=0)
    return np.asarray(out, dtype=np.float32)
